# revision 37
# baseline (speedup 1.0000x reference)
"""Causal depthwise Conv1d (K=4) for Trainium2, 8 NeuronCores.

Problem: x (B=8, L=4096, D=1024) f32, w (D, 1, 4), b (D,)
  y[n, l, d] = sum_k w[d, 0, k] * x[n, l - 3 + k, d] + b[d]   (zero pad l<0)

Sharding: data-parallel over batch — core i computes batch item i.

fp16 end-to-end design (gate is rel_err < 2e-2; this keeps ~4e-4):
  1. DMA natural [128_l, D] superblocks in as fp16 (halves HBM traffic),
     split into D-halves so compute starts after the first half lands.
  2. PE transposes 128x128 blocks into channels-on-partitions PSUM
     (fp16 transpose = 1 cyc/row); DVE/ACT materialize a 3-column-haloed
     SBUF tile (DVE reads psum-fp16 at 2x; lead groups per half use DVE).
  3. The 4-tap MAC is folded into the transpose-BACK matmuls: for taps
     k in taps_pe the rhs is diag(w_k) (built on-device by the otherwise
     idle GpSimd) and the stationary operand is the haloed xt shifted by
     k, accumulating y in natural layout in PSUM. Taps 2,3 + bias are
     computed by DVE tensor_scalar/tensor_tensor (2x fp16 mode) into an
     fp16 partial, merged by one more accumulating matmul with an
     identity rhs. LDWEIGHTS fully overlaps MATMUL on HW.
  4. ACT copies PSUM f32 -> SBUF fp16; per-half out-DMAs drain early.

Measured (NTFF, core 0): ~88.5-91 us vs 211.9 us baseline (~2.35x).
Engine busy: PE ~71us (95% mid-run), DVE ~70us, ACT ~53us; DMA engines
run in parallel on HW (~45% each) and are not the bottleneck.
"""

import sys
import types

import numpy as np

try:  # the NTFF profile hook module is absent in some containers
    import antenv.axon_hooks  # noqa: F401
except Exception:
    _stub = types.ModuleType("antenv.axon_hooks")
    _stub.get_axon_ntff_profile_hook = lambda: None
    try:
        import antenv

        sys.modules["antenv.axon_hooks"] = _stub
        antenv.axon_hooks = _stub
    except Exception:
        _pkg = types.ModuleType("antenv")
        _pkg.axon_hooks = _stub
        sys.modules["antenv"] = _pkg
        sys.modules["antenv.axon_hooks"] = _stub

import concourse.bass as bass
import concourse.bacc as bacc
import concourse.mybir as mybir
from concourse.tile import TileContext
from concourse.bass_utils import run_bass_kernel_spmd

P = 128
B = 8
L = 4096
D = 1024
K = 4
SB = 512  # L-superblock

CFG = {
    # per-group tap split: taps {0,1,2} partitioned over PE (matmul w/ diag
    # rhs) and DVE (TS mul + TT add). Tap 3 + bias is always the DVE
    # tensor_scalar that initializes the partial. (Pool/GpSimd cannot touch
    # PSUM and has no TensorScalarPtr on HW; it only gets SBUF TT/copies.)
    "taps_pe":   ((0, 1),) * 8,
    "taps_dve":  ((2,),) * 8,
    "taps_pool": ((),) * 8,
    "dve_second": "ts_tt",   # "ts_tt" | "stt" for DVE taps
    "xt_bufs": 3,
    "part_bufs": 2,
    "sb": 1024,
    # xt-copy engine per group g: "dve" | "act"  (DVE reads psum-fp16 at 2x;
    # lead groups per half get the faster DVE copy)
    "xt_copy": ("dve", "act", "act", "dve", "dve", "act", "act", "dve"),
    # out-copy engine per (t*2+h): "act" | "dve"
    "out_copy": ("act",) * 8,
    "split_in": True,    # per-half in-DMAs (earlier compute start)
    "split_out": True,   # per-half out-DMAs (earlier drain)
    "wdiag_pool": True,  # build diag weights on idle GpSimd
    "halo_pool": True,   # halo copies on idle GpSimd
}

ALU = mybir.AluOpType


def build_conv_nc(l=L, d=D, sb=None, reps=1):
    if sb is None:
        sb = CFG["sb"]
    G = d // P
    TPB = sb // P
    NSB = l // sb
    HD = d // 2
    GH = G // 2
    f32 = mybir.dt.float32
    f16 = mybir.dt.float16

    taps_pe = CFG["taps_pe"]
    taps_dve = CFG["taps_dve"]
    taps_pool = CFG["taps_pool"]
    pe_tap_set = sorted({k for taps in taps_pe for k in taps})

    nc = bacc.Bacc("TRN2", target_bir_lowering=False)
    x_d = nc.dram_tensor("x", [l, d], f16, kind="ExternalInput")
    ident_d = nc.dram_tensor("ident", [P, P], f16, kind="ExternalInput")
    wcols_d = nc.dram_tensor("wcols", [P, G * K], f32, kind="ExternalInput")
    bcol_d = nc.dram_tensor("bcol", [P, G], f32, kind="ExternalInput")
    y_dt = f16 if CFG.get("psum_out_dtype", "f32") == "f16" or not CFG.get("direct_out", False) else f32
    y_d = nc.dram_tensor("y", [l, d], y_dt, kind="ExternalOutput")

    with TileContext(nc) as tc:
        with (
            tc.tile_pool(name="const", bufs=1) as constp,
            tc.tile_pool(name="xin", bufs=CFG.get("xin_bufs", 3)) as xinp,
            tc.tile_pool(name="xt", bufs=CFG["xt_bufs"]) as xtp,
            tc.tile_pool(name="part", bufs=CFG["part_bufs"]) as partp,
            tc.tile_pool(name="tmp", bufs=2) as tmpp,
            tc.tile_pool(name="yout", bufs=CFG.get("yout_bufs", 2)) as youtp,
            tc.tile_pool(name="ps_in", bufs=CFG.get("psin_bufs", 4), space="PSUM") as psin,
            tc.tile_pool(name="ps_out", bufs=CFG.get("psout_bufs", 4), space="PSUM") as psout,
        ):
            x_r = x_d[:, :].rearrange("(s t p) d -> s p t d", p=P, t=TPB)
            y_r = y_d[:, :].rearrange("(s t p) d -> s p t d", p=P, t=TPB)

            # DMA order: first x superblock ahead of everything, then the
            # small consts (needed slightly later in the pipeline).
            def load_x(s):
                xt_ = xinp.tile([P, TPB, d], f16, tag="x", name="x_tile")
                sp = CFG.get("split_in", False)
                nsp = 2 if sp is True else (int(sp) if sp else 1)
                if nsp > 1:
                    w_ = d // nsp
                    for hh in range(nsp):
                        nc.sync.dma_start(
                            out=xt_[:, :, hh * w_ : (hh + 1) * w_],
                            in_=x_r[s][:, :, hh * w_ : (hh + 1) * w_],
                        )
                else:
                    nc.sync.dma_start(out=xt_, in_=x_r[s])
                return xt_

            x_tiles = {}
            x_tiles[0] = load_x(0)

            ident = constp.tile([P, P], f16)
            nc.sync.dma_start(out=ident, in_=ident_d[:, :])
            wcols = constp.tile([P, G * K], f32)
            nc.sync.dma_start(out=wcols, in_=wcols_d[:, :])
            bcol = constp.tile([P, G], f32)
            nc.sync.dma_start(out=bcol, in_=bcol_d[:, :])

            # build the diagonal weight matrices on-device during the DMA
            # ramp: wdiag[:, g*K+k, :] = ident * w_k (per-partition scalar)
            wdiag = constp.tile([P, G * K, P], f16)
            for g in range(G):
                for k in taps_pe[g]:
                    if CFG.get("wdiag_pool", False):
                        nc.gpsimd.tensor_tensor(
                            out=wdiag[:, g * K + k, :],
                            in0=ident[:, :],
                            in1=wcols[:, g * K + k : g * K + k + 1].broadcast_to(
                                [P, P]
                            ),
                            op=ALU.mult,
                        )
                    else:
                        nc.vector.tensor_scalar_mul(
                            wdiag[:, g * K + k, :],
                            ident[:, :],
                            wcols[:, g * K + k : g * K + k + 1],
                        )

            if NSB > 1:
                x_tiles[1] = load_x(1)

            import contextlib

            loop_cm = (
                tc.For_i(0, reps, 1, hint_engines=(mybir.EngineType.PE,))
                if reps > 1
                else contextlib.nullcontext()
            )
            prev_xt = [None] * G
            with loop_cm:
              for s in range(NSB):
                x_tile = x_tiles.pop(s)
                # prefetch 2 superblocks ahead of this one's out-DMA
                if s + 2 < NSB:
                    x_tiles[s + 2] = load_x(s + 2)

                xts = [None] * G
                parts = [None] * G
                y_tile = youtp.tile([P, TPB, d], f16)
                for h in range(2):
                  for g in range(h * GH, (h + 1) * GH):
                    # transpose-in: [128_l, 128_d] blocks -> [128_d, SB_l] psum
                    xt_ps = psin.tile([P, sb], f16)
                    for t in range(TPB):
                        nc.tensor.transpose(
                            xt_ps[:, t * P : (t + 1) * P],
                            x_tile[:, t, g * P : (g + 1) * P],
                            ident,
                        )
                    # haloed SBUF tile: cols [0,3) = previous superblock tail
                    xt = xtp.tile([P, K - 1 + sb], f16, tag=f"xt{g}")
                    if s == 0:
                        nc.vector.memset(xt[:, 0 : K - 1], 0.0)
                    elif CFG.get("halo_pool", False):
                        nc.gpsimd.tensor_copy(
                            out=xt[:, 0 : K - 1],
                            in_=prev_xt[g][:, sb : sb + K - 1],
                        )
                    else:
                        nc.vector.tensor_copy(
                            out=xt[:, 0 : K - 1],
                            in_=prev_xt[g][:, sb : sb + K - 1],
                        )
                    ceng = CFG["xt_copy"][g]
                    if ceng == "dve":
                        nc.vector.tensor_copy(out=xt[:, K - 1 :], in_=xt_ps[:, :])
                    else:
                        nc.scalar.copy(out=xt[:, K - 1 :], in_=xt_ps[:, :])
                    prev_xt[g] = xt
                    xts[g] = xt

                    # partial: tap3 (+bias) on DVE, then Pool/DVE taps
                    part = partp.tile([P, sb], f16, tag=f"part{g}")
                    nc.vector.tensor_scalar(
                        out=part[:, :],
                        in0=xt[:, 3 : 3 + sb],
                        scalar1=wcols[:, g * K + 3 : g * K + 4],
                        scalar2=bcol[:, g : g + 1],
                        op0=ALU.mult,
                        op1=ALU.add,
                    )
                    for k in taps_pool[g]:
                        nc.gpsimd.scalar_tensor_tensor(
                            out=part[:, :],
                            in0=xt[:, k : k + sb],
                            scalar=wcols[:, g * K + k : g * K + k + 1],
                            in1=part[:, :],
                            op0=ALU.mult,
                            op1=ALU.add,
                        )
                    for k in taps_dve[g]:
                        if CFG["dve_second"] == "ts_pooladd":
                            tmp = tmpp.tile([P, sb], f16, tag=f"tmp{g}")
                            nc.vector.tensor_scalar_mul(
                                tmp[:, :],
                                xt[:, k : k + sb],
                                wcols[:, g * K + k : g * K + k + 1],
                            )
                            nc.gpsimd.tensor_tensor(
                                out=part[:, :],
                                in0=tmp[:, :],
                                in1=part[:, :],
                                op=ALU.add,
                            )
                        elif CFG["dve_second"] == "stt":
                            nc.vector.scalar_tensor_tensor(
                                out=part[:, :],
                                in0=xt[:, k : k + sb],
                                scalar=wcols[:, g * K + k : g * K + k + 1],
                                in1=part[:, :],
                                op0=ALU.mult,
                                op1=ALU.add,
                            )
                        else:
                            tmp = tmpp.tile([P, sb], f16, tag=f"tmp{g}")
                            nc.vector.tensor_scalar_mul(
                                tmp[:, :],
                                xt[:, k : k + sb],
                                wcols[:, g * K + k : g * K + k + 1],
                            )
                            nc.vector.tensor_tensor(
                                out=part[:, :],
                                in0=tmp[:, :],
                                in1=part[:, :],
                                op=ALU.add,
                            )
                    parts[g] = part

                  # transpose-back for this half: only needs groups of half h
                  for t in range(TPB):
                    y_ps = psout.tile(
                        [P, HD],
                        f16 if CFG.get("psum_out_dtype", "f32") == "f16" else f32,
                    )
                    for j in range(GH):
                        g = h * GH + j
                        blk = y_ps[:, j * P : (j + 1) * P]
                        for ki, k in enumerate(taps_pe[g]):
                            nc.tensor.matmul(
                                blk,
                                xts[g][:, t * P + k : t * P + k + P],
                                wdiag[:, g * K + k, :],
                                start=(ki == 0),
                                stop=False,
                            )
                        nc.tensor.matmul(
                            blk,
                            parts[g][:, t * P : (t + 1) * P],
                            ident,
                            start=(len(taps_pe[g]) == 0),
                            stop=True,
                        )
                    if CFG.get("direct_out", False):
                        nc.scalar.dma_start(
                            out=y_r[s][:, t, h * HD : (h + 1) * HD],
                            in_=y_ps[:, :],
                        )
                        continue
                    dst = y_tile[:, t, h * HD : (h + 1) * HD]
                    oeng = CFG["out_copy"][(t * 2 + h) % len(CFG["out_copy"])]
                    if oeng == "act":
                        nc.scalar.copy(out=dst, in_=y_ps[:, :])
                    else:
                        nc.vector.tensor_copy(out=dst, in_=y_ps[:, :])
                  if CFG.get("direct_out", False):
                    pass
                  elif CFG.get("split_out", False):
                    nc.sync.dma_start(
                        out=y_r[s][:, :, h * HD : (h + 1) * HD],
                        in_=y_tile[:, :, h * HD : (h + 1) * HD],
                    )
                if not CFG.get("split_out", False) and not CFG.get("direct_out", False):
                    nc.sync.dma_start(out=y_r[s], in_=y_tile)
    nc.finalize()
    return nc


def host_prep(w, b):
    w = np.asarray(w, dtype=np.float32).reshape(D, K)
    b = np.asarray(b, dtype=np.float32).reshape(D)
    G = D // P
    wcols = np.empty((P, G * K), dtype=np.float32)
    bcol = np.empty((P, G), dtype=np.float32)
    for g in range(G):
        bcol[:, g] = b[g * P : (g + 1) * P]
        for k in range(K):
            wcols[:, g * K + k] = w[g * P : (g + 1) * P, k]
    ident = np.eye(P, dtype=np.float16)
    return {"wcols": wcols, "bcol": bcol, "ident": ident}


_NC_CACHE = {}


def _get_nc():
    key = (L, D, CFG["sb"])
    if key not in _NC_CACHE:
        _NC_CACHE[key] = build_conv_nc()
    return _NC_CACHE[key]


def kernel(x, w, b, _trace=False):
    x = np.asarray(x, dtype=np.float32)
    assert x.shape == (B, L, D), x.shape
    consts = host_prep(w, b)
    nc = _get_nc()
    x16 = x.astype(np.float16)
    in_maps = [{"x": np.ascontiguousarray(x16[i]), **consts} for i in range(B)]
    res = run_bass_kernel_spmd(nc, in_maps, core_ids=list(range(B)), trace=_trace)
    y = np.stack(
        [np.asarray(res.results[i]["y"]).astype(np.float32) for i in range(B)],
        axis=0,
    )
    if _trace:
        return y, res
    return y
